# revision 54
# baseline (speedup 1.0000x reference)
"""BitLinear (1-bit packed weights) on 8 TRN2 NeuronCores — v3.

out = x @ W.T, x [64, 4096] f32, W [11008, 4096] in {-1,+1} unpacked from
bp (one byte per int32, MSB-first bits).

Tensor-parallel over out_features: 8 x 1376 rows of W; x replicated.

v3 design (bitcast unpack):
 - DVE bitvec ops cannot dtype-cast (walrus verifier). Instead, the host
   repacks each weight bit into a bf16 EXPONENT bit position (9..14) of
   uint16 words. Then (word & (1<<s)) reinterpreted (bitcast) as bf16 is
   exactly {0, 2^(2^(s-7)-127)} — a clean one-op-per-plane unpack. The
   weird magnitude is compensated by prescaling the matching x slice by
   2^(128-2^(s-7)) on the host (products are always 2*x*bit).
 - 3 packed arrays btwA/B/C [128, 2752] u16: A carries 7 planes (pos
   15,14,13..9), B 6 (14..9), C 3 (14,13,12). Position 15 is extracted
   by the ACT engine via Sign (gives +-1 directly); the three pos-14
   planes go to GPSIMD tensor_tensor AND (one shared mask); the other 12
   ride the DVE (12 x ~1.04us chain).
 - "-1" of w=2b-1: rank-1 correction initializes psum via a
   contraction-2 matmul (c_hi+c_lo bf16 split for f32 accuracy).
 - Single large DMAs (packets spread over all 16 DMA engines), ungated.
 - u buffer holds all 16 planes: DVE never waits on PE.
 - PE: junk warmup for the p-state ramp, then corr + 16 planes in
   ascending-o order (matches producer completion), junk fillers between.
"""

import sys

sys.path.insert(0, "/opt/trn_rl_repo")

import ml_dtypes
import numpy as np

import concourse.bass as bass
import concourse.mybir as mybir
from concourse.bass_utils import run_bass_kernel_spmd

OUT_F = 11008
IN_F = 4096
M = 64
NCORES = 8
NSH = OUT_F // NCORES  # 1376 rows of W per core
NSH2 = 2 * NSH  # 2752
PACK = 16
NW = IN_F // PACK  # 256 words per W row
NCH = NW // 128  # 2 chunks
NPAIR = PACK  # 16 k-offsets per word
NA = NSH // 2  # 688
QS = (512, 176)  # psum n-splits per half

# plane table: k-offset o -> (array, bit position, producer)
# producers: 'dve', 'act' (Sign, pos must be 15), 'pool' (shared pos-14 mask)
PLANES = {
    0: ("A", 13, "dve"),
    1: ("A", 12, "dve"),
    2: ("A", 15, "sign"),
    3: ("A", 14, "dve"),
    4: ("A", 11, "dve"),
    5: ("A", 10, "dve"),
    6: ("A", 9, "dve"),
    7: ("B", 13, "dve"),
    8: ("B", 12, "dve"),
    9: ("B", 15, "sign"),
    10: ("B", 11, "dve"),
    11: ("B", 10, "dve"),
    12: ("B", 9, "dve"),
    13: ("C", 13, "dve"),
    14: ("C", 15, "sign"),
    15: ("C", 12, "dve"),
}
USE_SIGN = True  # ACT Sign for pos-15 planes; False -> DVE is_ge fallback

if not USE_SIGN:
    PLANES = {
        o: (a, p, "isge" if pr == "sign" else pr) for o, (a, p, pr) in PLANES.items()
    }
DVE_SEQ = [o for o in range(NPAIR) if PLANES[o][2] != "sign"]
SIGN_SEQ = [o for o in range(NPAIR) if PLANES[o][2] == "sign"]
# PE consumption order: sign planes deferred to match ACT completion times
PE_ORDER = [0, 1, 3, 4, 2, 5, 6, 7, 8, 9, 10, 11, 12, 13, 14, 15] if USE_SIGN else list(range(NPAIR))

N_WARM = 12
WARM_N = 256
FILL_N = 128
CCOL = NSH2  # c_hi/c_lo column block inside bpA


_np_word = "<u2"  # test.py preflight compat


def _shift(o):
    # ORIGINAL byte packing of bp: bit position of k-offset o
    return 8 * (o // 8) + 7 - (o % 8)


def _xscale(o):
    arr, pos, prod = PLANES[o]
    if prod == "sign":
        return 1.0  # Sign gives +-1 directly
    if prod == "isge":
        return 2.0  # is_ge gives {0,1}
    # u_on = 2^(2^(pos-7)-127); want x'*u_on = 2x
    return float(2.0 ** (128 - (1 << (pos - 7))))


def _build():
    nc = bass.Bass()
    bpA = nc.declare_dram_parameter("bpA", [128, NSH2 + 64], mybir.dt.uint16, isOutput=False)
    bpB = nc.declare_dram_parameter("bpB", [128, NSH2], mybir.dt.uint16, isOutput=False)
    bpC = nc.declare_dram_parameter("bpC", [128, NSH2], mybir.dt.uint16, isOutput=False)
    xr = nc.declare_dram_parameter("xr", [128, 2 * NPAIR * M], mybir.dt.bfloat16, isOutput=False)
    out = nc.declare_dram_parameter("out", [128, NA], mybir.dt.float32, isOutput=True)

    A = mybir.AluOpType
    AF = mybir.ActivationFunctionType
    BF = mybir.dt.bfloat16

    sv_idx = {o: i + 1 for i, o in enumerate(DVE_SEQ)}

    from contextlib import ExitStack

    with ExitStack() as stack:
        ec = stack.enter_context
        xb = ec(nc.sbuf_tensor("xb", [128, 2 * NPAIR * M], mybir.dt.bfloat16))
        btw = {
            "A": ec(nc.sbuf_tensor("btwA", [128, NSH2 + 64], mybir.dt.uint16)),
            "B": ec(nc.sbuf_tensor("btwB", [128, NSH2], mybir.dt.uint16)),
            "C": ec(nc.sbuf_tensor("btwC", [128, NSH2], mybir.dt.uint16)),
        }
        u = ec(nc.sbuf_tensor("u", [128, NPAIR, NSH2], mybir.dt.uint16))
        usg = ec(nc.sbuf_tensor("usg", [128, max(1, len(SIGN_SEQ)), NSH2], mybir.dt.bfloat16))
        ones2 = ec(nc.sbuf_tensor("ones2", [2, QS[0]], mybir.dt.bfloat16))
        ot2 = ec(nc.sbuf_tensor("ot2", [128, NA], mybir.dt.float32))
        junk = ec(nc.sbuf_tensor("junk", [128, WARM_N], mybir.dt.bfloat16))
        scr = ec(nc.sbuf_tensor("scr", [1, 1], mybir.dt.float32))
        q0 = ec(nc.psum_tensor("q0", [128, QS[0]], mybir.dt.float32))
        q1 = ec(nc.psum_tensor("q1", [128, QS[1]], mybir.dt.float32))
        psw = ec(nc.psum_tensor("psw", [M, WARM_N], mybir.dt.float32))
        sba = ec(nc.semaphore("sba"))
        sbb = ec(nc.semaphore("sbb"))
        sbc = ec(nc.semaphore("sbc"))
        sq0 = ec(nc.semaphore("sq0"))  # xr0 dma
        sx1 = ec(nc.semaphore("sx1"))  # xr rest dma
        sv = ec(nc.semaphore("sv"))  # DVE plane counter
        sa = ec(nc.semaphore("sa"))  # ACT: signA=1 signB=2 signC=3 drQ1=4 drQ0a=5
        sg = ec(nc.semaphore("sg"))  # gpsimd: sign-bias memset = 1
        sp = ec(nc.semaphore("sp"))  # PE: q1 stopped=1, q0 stopped=2
        so = ec(nc.semaphore("so"))  # out dma completions (2 x 16)
        bsgn = ec(nc.sbuf_tensor("bsgn", [128, 1], mybir.dt.float32))
        block = ec(nc.Block())

        # DMA facts (measured): rings start serially ~1.3us apart; each ring
        # bursts ~160GB/s on full-size row packets, degrading under
        # contention. So: ring1 = A then B (FIFO), ring2 = xr0, xr-rest, C.

        @block.sync
        def _(sync: bass.BassEngine):
            sync.dma_start(out=btw["A"][:, :], in_=bpA[:, :]).then_inc(sba, 16)
            sync.dma_start(out=btw["B"][:, :], in_=bpB[:, :]).then_inc(sbb, 16)
            sync.wait_ge(sa, len(SIGN_SEQ) + 1)
            sync.dma_start(out=out[:, QS[0]:NA], in_=ot2[:, QS[0]:NA]).then_inc(so, 16)
            sync.wait_ge(sa, len(SIGN_SEQ) + 2)
            sync.dma_start(out=out[:, 256:QS[0]], in_=ot2[:, 256:QS[0]]).then_inc(so, 16)
            sync.wait_ge(so, 48)

        @block.vector
        def _(vector: bass.BassEngine):
            vector.wait_ge(sba, 16)
            seen = set()
            for o in DVE_SEQ:
                arr, pos, prod = PLANES[o]
                if arr == "B" and "B" not in seen:
                    vector.wait_ge(sbb, 16)
                if arr == "C" and "C" not in seen:
                    vector.wait_ge(sbc, 16)
                seen.add(arr)
                if prod == "isge":
                    vector.tensor_scalar(
                        u[:, o, :].bitcast(BF), btw[arr][:, 0:NSH2], 32768.0, None,
                        op0=A.is_ge,
                    ).then_inc(sv)
                else:
                    vector.tensor_scalar(
                        u[:, o, :], btw[arr][:, 0:NSH2], 1 << pos, None,
                        op0=A.bitwise_and,
                    ).then_inc(sv)


        @block.scalar
        def _(scalar: bass.BassEngine):
            scalar.dma_start(out=xb[:, 0:512], in_=xr[:, 0:512]).then_inc(sq0, 16)
            scalar.dma_start(out=xb[:, 512:2048], in_=xr[:, 512:2048]).then_inc(sx1, 16)
            scalar.dma_start(out=btw["C"][:, :], in_=bpC[:, :]).then_inc(sbc, 16)
            scalar.activation(scr[:, :], scr[:, :], AF.Copy, 0.0, 0.0)  # table prime
            scalar.wait_ge(sg, 1)
            for i, o in enumerate(SIGN_SEQ):
                arr, pos, _ = PLANES[o]
                scalar.wait_ge({"A": sba, "B": sbb, "C": sbc}[arr], 16)
                scalar.activation(
                    usg[:, i, :], btw[arr][:, 0:NSH2], AF.Sign,
                    bias=bsgn[:, 0:1], scale=1.0,
                ).then_inc(sa)
            scalar.wait_ge(sp, 1)
            scalar.activation(
                ot2[:, QS[0]:NA], q1[:, :], AF.Copy, bias=0.0, scale=1.0
            ).then_inc(sa)
            scalar.wait_ge(sp, 2)
            scalar.activation(
                ot2[:, 0:QS[0]], q0[:, :], AF.Copy, bias=0.0, scale=1.0
            ).then_inc(sa)
            # wait for our own drain's retire: dma_start is seq-only and would
            # otherwise race the ACT engine pipeline
            scalar.wait_ge(sa, len(SIGN_SEQ) + 2)
            scalar.dma_start(out=out[:, 0:256], in_=ot2[:, 0:256]).then_inc(so, 16)

        @block.gpsimd
        def _(gpsimd: bass.BassEngine):
            gpsimd.memset(bsgn[:, :], -32767.5).then_inc(sg)  # sg=1
            gpsimd.memset(ones2[:, :], 1.0).then_inc(sg)  # sg=2

        @block.tensor
        def _(tensor: bass.BassEngine):
            for _i in range(N_WARM):
                tensor.matmul(psw[:, :], junk[:, 0:M], junk[:, :], start=True, stop=True)
            # rank-1 correction initializes psum: psum[m, n] = c_hi[m] + c_lo[m]
            tensor.wait_ge(sg, 2)
            tensor.wait_ge(sba, 16)
            chl = btw["A"][0:2, NSH2 : NSH2 + 64].bitcast(BF)
            for tp, pbase in (((0, 0), 0), ((0, 64), 64)):
                tensor.matmul(
                    q0[pbase:pbase + M, :], chl, ones2[0:2, 0:QS[0]],
                    start=True, stop=False, tile_position=tp,
                )
                tensor.matmul(
                    q1[pbase:pbase + M, :], chl, ones2[0:2, 0:QS[1]],
                    start=True, stop=False, tile_position=tp,
                )
            for _k in range(2):
                tensor.matmul(
                    psw[:, 0:FILL_N], junk[:, 0:M], junk[:, 0:FILL_N],
                    start=True, stop=True,
                )
            sa_idx = {o: i + 1 for i, o in enumerate(SIGN_SEQ)}
            for idx, o in enumerate(PE_ORDER):
                xr_sem = sq0 if o < 4 else sx1
                tensor.wait_ge(xr_sem, 16)
                is_sign = PLANES[o][2] == "sign"
                if is_sign:
                    tensor.wait_ge(sa, sa_idx[o])
                else:
                    tensor.wait_ge(sv, sv_idx[o])
                is_last = idx == NPAIR - 1
                for c in range(NCH):
                    lh = xb[:, (o * 2 + c) * M : (o * 2 + c + 1) * M]
                    tensor.ldweights(lh, tile_position=(0, 0))
                    tensor.ldweights(lh, tile_position=(0, 64))
                    base = c * NSH
                    lc = is_last and c == NCH - 1
                    splits = [(q1, QS[0], QS[1]), (q0, 0, QS[0])] if lc else [
                        (q0, 0, QS[0]), (q1, QS[0], QS[1])]
                    for qt, off, w in splits:
                        for tp, pbase, nbase in (((0, 0), 0, 0), ((0, 64), 64, NA)):
                            lo = base + nbase + off
                            rhs = (
                                usg[:, sa_idx[o] - 1, lo : lo + w]
                                if is_sign
                                else u[:, o, lo : lo + w].bitcast(BF)
                            )
                            mm = tensor.matmul(
                                qt[pbase:pbase + M, :],
                                lh,
                                rhs,
                                start=False, stop=lc,
                                tile_position=tp,
                            )
                            mm.ins.ldweights = False
                            if lc and tp == (0, 64):
                                mm.then_inc(sp)
                if idx < 4:
                    tensor.matmul(
                        psw[:, 0:FILL_N], junk[:, 0:M], junk[:, 0:FILL_N],
                        start=True, stop=True,
                    )

    return nc


def _prep(x, bp):
    x = np.asarray(x, dtype=np.float32)
    bp = np.asarray(bp)
    bytes_ = bp.astype(np.uint8)
    B = bytes_.reshape(OUT_F, IN_F // 8)

    # x[m, k] with k = PACK*(128*c + p) + o  ->  xh[p, (o, c, m)], prescaled
    xr4 = np.ascontiguousarray(
        x.reshape(M, NCH, 128, PACK).transpose(2, 3, 1, 0)
    )  # [p, o, c, m]
    for o in range(PACK):
        xr4[:, o, :, :] *= _xscale(o)
    xh = xr4.reshape(128, -1).astype(ml_dtypes.bfloat16)

    # rank-1 correction: {0, u_on}-form planes need -sum(x) over their k's;
    # Sign planes (+-1 form) need none.
    ks = x.reshape(M, NW, PACK)
    sign_sum = sum((ks[:, :, o].sum(axis=1) for o in SIGN_SEQ), np.zeros(M, np.float32))
    corr = -(x.sum(axis=1) - sign_sum)  # [M]
    c_hi = corr.astype(ml_dtypes.bfloat16)
    c_lo = (corr - c_hi.astype(np.float32)).astype(ml_dtypes.bfloat16)
    # c_hi/c_lo ride in bpA's extra columns (rows 0/1) as bf16 bit patterns
    cblk = np.zeros((128, 64), np.uint16)
    cblk[0, :] = c_hi.view(np.uint16)
    cblk[1, :] = c_lo.view(np.uint16)

    in_maps = []
    for cid in range(NCORES):
        Bc = np.ascontiguousarray(B[cid * NSH : (cid + 1) * NSH])  # [1376, 512] u8
        Wd = Bc.view("<u2")  # [1376, NW] little-endian words
        bptT = np.ascontiguousarray(Wd.T)  # [256, 1376], row w = 128c+p
        # bit o of word -> new array/position
        packs = {"A": np.zeros((256, NSH), np.uint16),
                 "B": np.zeros((256, NSH), np.uint16),
                 "C": np.zeros((256, NSH), np.uint16)}
        for o in range(PACK):
            arr, pos, _ = PLANES[o]
            bit = (bptT >> np.uint16(_shift(o))) & np.uint16(1)
            packs[arr] |= bit << np.uint16(pos)
        im = {}
        for name, arrk in (("bpA", "A"), ("bpB", "B"), ("bpC", "C")):
            full = packs[arrk]
            pair = np.concatenate([full[0:128, :], full[128:256, :]], axis=1)
            if arrk == "A":
                pair = np.concatenate([pair, cblk], axis=1)
            im[name] = np.ascontiguousarray(pair)
        im["xr"] = xh
        in_maps.append(im)
    return in_maps


def _run(x, bp, trace=False):
    in_maps = _prep(x, bp)
    nc = _build()
    res = run_bass_kernel_spmd(nc, in_maps, list(range(NCORES)), trace=trace)
    outs = []
    for c in range(NCORES):
        o = np.asarray(res.results[c]["out"])  # [128, 688]
        outs.append(np.concatenate([o[0:M, :], o[M:128, :]], axis=1))  # [64, 1376]
    full = np.concatenate(outs, axis=1).astype(np.float32)
    return full, res


def kernel(x, bp):
    out, _ = _run(x, bp, trace=False)
    return out
